# revision 1
# baseline (speedup 1.0000x reference)
import math
from functools import partial

import numpy as np
import jax
import jax.numpy as jnp

# Problem constants (nn_Gated_MultiHeadSelfAttention): hardcoded per spec.
B, N, E, H, DPOS = 64, 196, 768, 16, 3
DH = E // H
NCORES = 8
BLOC = B // NCORES  # 8 batches per core, data-parallel over batch


def _rel_pos(num_patches: int) -> np.ndarray:
    v = int(math.isqrt(num_patches))
    ind = np.arange(v)[None, :] - np.arange(v)[:, None]
    indx = np.tile(ind, (v, v))
    indy = np.repeat(np.repeat(ind, v, axis=0), v, axis=1)
    indd = indx ** 2 + indy ** 2
    return np.stack([indx, indy, indd], axis=-1).astype(np.float32)


@partial(
    jax.pmap,
    axis_name="dp",
    in_axes=(0, 0, 0, 0, None, None, None, None, None, None, None),
)
def _fwd(q, k, v, mask, Wq, Wk, Wv, Wo, bo, gating, pos_sm):
    b, n, _ = q.shape

    def heads(x, W):  # [b, n, E] -> [b, H, n, DH]
        return (x @ W.T).reshape(b, n, H, DH).transpose(0, 2, 1, 3)

    Q, K, V = heads(q, Wq), heads(k, Wk), heads(v, Wv)

    energy = jnp.einsum("bhqd,bhkd->bhqk", Q, K) / math.sqrt(DH)
    energy = jnp.where(mask, energy, -jnp.inf)
    content = jax.nn.softmax(energy, axis=-1)

    g = jax.nn.sigmoid(gating)[None, :, None, None]
    score = (1.0 - g) * content + g * pos_sm[None]
    score = score / jnp.sum(score, axis=-1, keepdims=True)

    attn = jnp.einsum("bhqk,bhkd->bhqd", score, V)
    out = attn.transpose(0, 2, 1, 3).reshape(b, n, E) @ Wo.T + bo
    return out


def kernel(q, k, v, mask, Wq, Wk, Wv, Wo, bo, Wpos, bpos, gating):
    # Positional branch is batch-independent: compute once on host, replicate.
    rel = _rel_pos(N)                                   # [N, N, 3]
    pos = np.einsum("qkp,hp->hqk", rel, np.asarray(Wpos)) + np.asarray(bpos)[:, None, None]
    pos = pos - pos.max(axis=-1, keepdims=True)
    pos = np.exp(pos)
    pos_sm = (pos / pos.sum(axis=-1, keepdims=True)).astype(np.float32)  # [H, N, N]

    qs = np.asarray(q, np.float32).reshape(NCORES, BLOC, N, E)
    ks = np.asarray(k, np.float32).reshape(NCORES, BLOC, N, E)
    vs = np.asarray(v, np.float32).reshape(NCORES, BLOC, N, E)
    ms = np.asarray(mask).reshape(NCORES, BLOC, 1, N, N)

    out = _fwd(
        qs, ks, vs, ms,
        jnp.asarray(Wq, jnp.float32), jnp.asarray(Wk, jnp.float32),
        jnp.asarray(Wv, jnp.float32), jnp.asarray(Wo, jnp.float32),
        jnp.asarray(bo, jnp.float32), jnp.asarray(gating, jnp.float32),
        jnp.asarray(pos_sm),
    )
    return np.asarray(out).reshape(B, N, E).astype(np.float32)
